# revision 2
# baseline (speedup 1.0000x reference)
"""APT encoder scatter kernel for TRN2 (8 NeuronCores, data-parallel over batch).

Problem: scatter patch tokens [B, P*BS, D] to a dense grid [B, H, W, T, BS, D]
per positions [B, P, 4] (rows y, x, size, t), broadcasting size-2 patches over
their 2x2 cell footprint.

Design (per core, one sample):
  - out row index for a cell (y, x, t) is 128*y + 4*x + t (H=W=32, T=4); each
    row is BS*D = 2304 f32 = 9216 B.
  - For each patch, its up-to-4 destination rows are pure elementwise int math
    on the position row: off_j = 128*(y+dy_j) + 4*(x+dx_j) + t, valid iff
    dy_j < size and dx_j < size (slot 0 always valid, slots 1..3 iff size==2).
    Invalid slots get an out-of-bounds row id and are silently skipped by the
    indirect DMA's bounds check.
  - Pipeline: 20 tiles of 128 patches (partition p of tile i = patch 20p+i);
    each tile is one HWDGE load [128, 2304] plus 4 SWDGE indirect scatters
    into the output, offsets read from a precomputed [128, 80] int32 tile.
"""

import numpy as np

import concourse.bass as bass
import concourse.bacc as bacc
import concourse.mybir as mybir
import concourse.tile as tile
from concourse.bass_utils import run_bass_kernel_spmd

B = 8
H, W, T, BS, D = 32, 32, 4, 3, 768
P = 2560             # patches per sample
ROW = BS * D         # 2304 f32 per patch/cell row
NCELL = H * W * T    # 4096 output rows per sample
PPART = 20           # patches per SBUF partition (2560 = 128 * 20)
NT = PPART           # scatter tiles per sample
OOB = 8192           # out-of-bounds row sentinel (> NCELL-1): skipped by DMA

_CACHE = {}


def _build():
    nc = bacc.Bacc("TRN2", target_bir_lowering=False, debug=False, num_devices=B)
    tok = nc.declare_dram_parameter("tok", [P, ROW], mybir.dt.float32, isOutput=False)
    pos = nc.declare_dram_parameter("pos", [P, 4], mybir.dt.int32, isOutput=False)
    out = nc.declare_dram_parameter("out", [NCELL, ROW], mybir.dt.float32, isOutput=True)

    i32 = mybir.dt.int32
    f32 = mybir.dt.float32
    Op = mybir.AluOpType

    with tile.TileContext(nc) as tc:
        with (
            tc.tile_pool(name="meta", bufs=1) as meta,
            tc.tile_pool(name="toks", bufs=4) as toks,
        ):
            # partition p holds patches [20p, 20p+19], 4 ints each (contiguous)
            pos_sb = meta.tile([128, PPART * 4], i32)
            nc.sync.dma_start(
                out=pos_sb[:], in_=pos[:].rearrange("(p i) c -> p (i c)", p=128)
            )

            pos3 = pos_sb[:].rearrange("p (i c) -> p i c", c=4)
            y = pos3[:, :, 0]
            x = pos3[:, :, 1]
            s = pos3[:, :, 2]
            t = pos3[:, :, 3]

            base = meta.tile([128, PPART], i32)
            notbig = meta.tile([128, PPART], i32)
            offs = meta.tile([128, PPART * 4], i32)
            offs3 = offs[:].rearrange("p (i c) -> p i c", c=4)

            nc.vector.tensor_scalar(
                out=base[:], in0=y, scalar1=128, scalar2=None, op0=Op.mult
            )
            nc.vector.scalar_tensor_tensor(
                out=base[:], in0=x, scalar=4, in1=base[:], op0=Op.mult, op1=Op.add
            )
            nc.vector.tensor_tensor(out=base[:], in0=base[:], in1=t, op=Op.add)
            # 0 where size==2 (slots 1..3 valid), OOB where size==1
            nc.vector.tensor_scalar(
                out=notbig[:], in0=s, scalar1=2, scalar2=OOB, op0=Op.is_lt, op1=Op.mult
            )

            nc.vector.tensor_copy(offs3[:, :, 0], base[:])
            for j, cj in ((1, 4), (2, 128), (3, 132)):
                nc.vector.scalar_tensor_tensor(
                    out=offs3[:, :, j],
                    in0=base[:],
                    scalar=cj,
                    in1=notbig[:],
                    op0=Op.add,
                    op1=Op.add,
                )

            tok_r = tok[:].rearrange("(p i) r -> p i r", i=PPART)
            for i in range(NT):
                tok_t = toks.tile([128, ROW], f32)
                nc.sync.dma_start(out=tok_t[:], in_=tok_r[:, i, :])
                for j in range(4):
                    col = 4 * i + j
                    nc.gpsimd.indirect_dma_start(
                        out=out[:],
                        out_offset=bass.IndirectOffsetOnAxis(
                            ap=offs[:, col : col + 1], axis=0
                        ),
                        in_=tok_t[:],
                        in_offset=None,
                        bounds_check=NCELL - 1,
                        oob_is_err=False,
                    )

    nc.compile()
    return nc


def _run(modality_tokens, positions, trace=False, tmpdir=None):
    nc = _CACHE.get("nc")
    if nc is None:
        nc = _CACHE["nc"] = _build()
    toks = np.ascontiguousarray(np.asarray(modality_tokens, dtype=np.float32)).reshape(
        B, P, ROW
    )
    poss = np.ascontiguousarray(np.asarray(positions, dtype=np.int32))
    in_maps = [{"tok": toks[b], "pos": poss[b]} for b in range(B)]
    res = run_bass_kernel_spmd(
        nc, in_maps, core_ids=list(range(B)), trace=trace, tmpdir=tmpdir
    )
    outf = np.stack([res.results[b]["out"] for b in range(B)])
    return outf.reshape(B, H, W, T, BS, D), res


def kernel(modality_tokens, positions):
    outf, _ = _run(modality_tokens, positions)
    return outf
